# revision 34
# baseline (speedup 1.0000x reference)
# Trainium2 Bass kernel for nn_DepthCorr (SiamRPN-style depthwise correlation head).
#
# Pipeline (per batch):
#   kf   = relu(bn(conv3x3(kernel, Wk)))   [C=256, 7,7]  -> [H=256, 5,5]
#   sf   = relu(bn(conv3x3(search, Ws)))   [C=256,31,31] -> [H=256,29,29]
#   corr = relu(dwxcorr(sf, kf))                         -> [H=256,25,25]
#   out  = relu(bn(conv1x1(corr, Wf)))                   -> [C=256,25,25]
#
# Sharding: pure data-parallel over batch (128 batches / 8 cores = 16 per core).
# BN is folded into conv weights + per-channel bias on the host.
#
# Engine split: the depthwise xcorr's 25 taps are spread across three engines
# (PE diag-matmuls ~0.56us/tap, DVE tensor_scalar(4x)+add(2x) ~1.39us/tap,
# ACT mults ~1.65us/tap + DVE adds 0.78us), tuned so PE/DVE/ACT land within
# ~1us/batch of each other.  GPSIMD does DMA only: its tensor ops run at
# ~2.6 cyc/elem AND their SBUF port traffic slows DVE ops ~3x and PE matmuls
# 10-25% (measured on HW), so it must never touch tensor work.
#
# conv2 runs 3 of its 9 taps as fp8e4 DoubleRow matmuls (contracting both
# 128-channel groups in one pass at 2 MACs/cell/cycle); measured end-to-end
# rel err 1.6e-2 against the fp32 reference (gate 2e-2), the rest fp16.
#
# The last batch runs 19 taps on the PE (plus 6 ACT taps) with its diag
# built right after conv1 (kf is available from batch 0): the tail otherwise
# stalls ~16us waiting for the last batch's DVE offload chain with no later
# batch to overlap it.
#
# Measured on trn2 (8 cores): 357-361us HW exec over 6 runs at the full
# 2.4GHz PE clock (vs 368.5us fp16 baseline), max rel err 1.46e-2.  Runs
# land ~5-15% higher when the chip sits in a lower P-state.
#
# Known remaining stall (~15-25us): conv2's PSUM banks recycle via its relu
# epilogues on the ACT queue, which sit behind the ~13us AV-copy block of
# the previous batch's offload, stalling the PE 1-4us on many batches.
# Moving the epilogues to GPSIMD (tensor_scalar add-bias/max-0 from PSUM)
# builds at the bass level but crashes the neuronxcc/walrus lowering
# (opaque INTERNAL error; suspect the per-partition AP scalar operand on
# Pool).  Tried and measured WORSE: hg-major offload emission (serial add
# tail), offload deferral by one iteration (copies serialize behind next
# conv2's relus), ps_c=4 with ps_o=1 or ps_x=2 (donor pool starves), DVE
# epilogues (DVE queue has the same blocking), bigger pool bufs, queue
# reassignments.
import numpy as np
from contextlib import ExitStack

import ml_dtypes

import concourse.mybir as mybir
import concourse.tile as tile
from concourse import bacc
from concourse.bass_utils import run_bass_kernel_spmd

B, C, H = 128, 256, 256
N_CORES = 8
NB = B // N_CORES  # batches per core
EPS = 1e-5
FP = mybir.dt.float32
RELU = mybir.ActivationFunctionType.Relu
COPY = mybir.ActivationFunctionType.Copy
F16 = mybir.dt.float16
F8 = mybir.dt.float8e4
DR = mybir.MatmulPerfMode.DoubleRow

PE_TAPS = [(1, 0), (1, 1), (1, 2), (1, 3), (1, 4),
           (2, 0), (2, 1), (2, 2), (2, 3)]
PE_T0 = 5  # first PE tap index (contiguous run t=5..13 in kf's tap dim)
PE_NRUN = 9  # taps in the contiguous run
# last batch: 19 taps on PE (indexed into the 25-tap diag_last) + 6 on ACT;
# its diag is prebuilt so only the short ACT+DVE acc chain trails conv2.
AV_LAST = [(0, 1), (0, 3), (3, 1), (3, 3), (4, 1), (4, 3)]
PE_LAST = [(ti, tj) for ti in range(5) for tj in range(5)
           if (ti, tj) not in AV_LAST]
# TS taps restricted to tj in {0,2,4}: with 64B sf rows these windows stay
# 4B/8B-aligned for the DVE 2x/4x tensor_scalar modes.
TS_TAPS = [(0, 2), (0, 4), (3, 0), (3, 2), (3, 4), (4, 0), (4, 2), (2, 4)]
AV_TAPS = [(0, 0), (0, 1), (0, 3), (3, 1), (3, 3), (4, 1), (4, 3), (4, 4)]
# conv2 taps computed in fp8 DoubleRow (both operands fp8e4).  dx=0 taps
# only: their moving operand is a full-width 32-col window, which coalesces
# to the 3-dim [Ki, Ko=2, dim] AP DoubleRow wants (cols 29..31 of the PSUM
# tile accumulate garbage that the epilogue never reads).  Precision sim:
# rel err ~1.6e-2 at k=3 (gate 2e-2).
FP8_TAPS = [(0, 0), (1, 0), (2, 0)]


def _build_nc(nb=NB):
    nc = bacc.Bacc()

    search = nc.declare_dram_parameter("search", [nb, C, 31, 32], F16, isOutput=False)
    search8 = nc.declare_dram_parameter("search8", [nb, C, 31, 32], F8, isOutput=False)
    # kin pre-transposed on the host to [k, cg, h, w, b] so the DMA is contiguous
    kin = nc.declare_dram_parameter("kin", [128, 2, 7, 7, nb], F16, isOutput=False)
    wk_d = nc.declare_dram_parameter("wk", [128, 36, 128], F16, isOutput=False)
    ws_d = nc.declare_dram_parameter("ws", [128, 36, 128], F16, isOutput=False)
    ws8_d = nc.declare_dram_parameter("ws8", [128, 6, 2, 128], F8, isOutput=False)
    wf_d = nc.declare_dram_parameter("wf", [128, 4, 128], F16, isOutput=False)
    bias_d = nc.declare_dram_parameter("bias", [128, 6], FP, isOutput=False)
    eye_d = nc.declare_dram_parameter("eye", [128, 128], F16, isOutput=False)
    out_d = nc.declare_dram_parameter("out", [nb, C, 25, 25], F16, isOutput=True)

    C2_SPLITS = [(0, 16), (16, 13)]  # conv2 row splits; N = 464 / 377
    O_SPLITS = [(0, 13), (13, 12)]  # conv3 row splits over corr; N = 325 / 300

    with tile.TileContext(nc) as tc, ExitStack() as ctx:
        wpool = ctx.enter_context(tc.tile_pool(name="wpool", bufs=1))
        kpool = ctx.enter_context(tc.tile_pool(name="kpool", bufs=1))
        spool = ctx.enter_context(tc.tile_pool(name="spool", bufs=3))
        s8pool = ctx.enter_context(tc.tile_pool(name="s8pool", bufs=3))
        fpool = ctx.enter_context(tc.tile_pool(name="fpool", bufs=5))
        dpool = ctx.enter_context(tc.tile_pool(name="dpool", bufs=3))
        apool = ctx.enter_context(tc.tile_pool(name="apool", bufs=4))
        cpool = ctx.enter_context(tc.tile_pool(name="cpool", bufs=2))
        tpool = ctx.enter_context(tc.tile_pool(name="tpool", bufs=10))
        opool = ctx.enter_context(tc.tile_pool(name="opool", bufs=2))
        ps_c = ctx.enter_context(tc.tile_pool(name="ps_c", bufs=3, space="PSUM"))
        ps_x = ctx.enter_context(tc.tile_pool(name="ps_x", bufs=3, space="PSUM"))
        ps_o = ctx.enter_context(tc.tile_pool(name="ps_o", bufs=2, space="PSUM"))

        # --- search prefetch (sync/scalar hwdge queues; big weights go on
        # the gpsimd swdge queue) ---
        s_tiles = {}

        def load_search(b):
            s_sb = spool.tile([128, 2, 31, 32], F16, tag="sin")
            s8_sb = s8pool.tile([128, 2, 31, 32], F8, tag="sin8")
            nc.sync.dma_start(out=s8_sb[:, 0, :, :], in_=search8[b, 0:128, :, :])
            nc.scalar.dma_start(out=s8_sb[:, 1, :, :], in_=search8[b, 128:256, :, :])
            nc.sync.dma_start(out=s_sb[:, 0, :, :], in_=search[b, 0:128, :, :])
            nc.scalar.dma_start(out=s_sb[:, 1, :, :], in_=search[b, 128:256, :, :])
            s_tiles[b] = (s_sb, s8_sb)

        # --- resident constants ---
        wk_sb = wpool.tile([128, 36, 128], F16, tag="wk")
        ws_sb = wpool.tile([128, 36, 128], F16, tag="ws")
        ws8_sb = wpool.tile([128, 6, 2, 128], F8, tag="ws8")
        wf_sb = wpool.tile([128, 4, 128], F16, tag="wf")
        bias_sb = wpool.tile([128, 6], FP, tag="bias")
        eye_sb = wpool.tile([128, 128], F16, tag="eye")
        # head staging: everything conv2(0)'s first matmuls need goes first
        # on the two hwdge queues (the scalar queue is blocked ~1.3us by the
        # activation-table load, the gpsimd swdge pays ~0.7us per DIRECT2D)
        s_sb0 = spool.tile([128, 2, 31, 32], F16, tag="sin")
        s8_sb0 = s8pool.tile([128, 2, 31, 32], F8, tag="sin8")
        nc.sync.dma_start(out=s_sb0[:, 0, 0:19, :], in_=search[0, 0:128, 0:19, :])
        nc.scalar.dma_start(out=bias_sb[:], in_=bias_d[:])
        nc.scalar.dma_start(out=s_sb0[:, 1, 0:19, :], in_=search[0, 128:256, 0:19, :])
        nc.gpsimd.dma_start(out=ws_sb[:, 2:6, :], in_=ws_d[:, 2:6, :])
        nc.sync.dma_start(out=ws_sb[:, 8:12, :], in_=ws_d[:, 8:12, :])
        nc.scalar.dma_start(out=ws8_sb[:], in_=ws8_d[:])
        nc.scalar.dma_start(out=ws_sb[:, 14:18, :], in_=ws_d[:, 14:18, :])
        nc.sync.dma_start(out=s8_sb0[:, 0, :, :], in_=search8[0, 0:128, :, :])
        nc.scalar.dma_start(out=s8_sb0[:, 1, :, :], in_=search8[0, 128:256, :, :])
        nc.sync.dma_start(out=s_sb0[:, 0, 19:31, :], in_=search[0, 0:128, 19:31, :])
        nc.scalar.dma_start(out=s_sb0[:, 1, 19:31, :], in_=search[0, 128:256, 19:31, :])
        nc.gpsimd.dma_start(out=ws_sb[:, 20:24, :], in_=ws_d[:, 20:24, :])
        nc.gpsimd.dma_start(out=ws_sb[:, 26:30, :], in_=ws_d[:, 26:30, :])
        nc.gpsimd.dma_start(out=ws_sb[:, 32:36, :], in_=ws_d[:, 32:36, :])
        nc.scalar.dma_start(out=eye_sb[:], in_=eye_d[:])
        s_tiles[0] = (s_sb0, s8_sb0)
        k_sbs = []
        for cg in range(2):
            k_sb = kpool.tile([128, 7, 7, nb], F16, tag=f"kin{cg}")
            k_sbs.append(k_sb)
        # kf_sb[h_part, hg, tap, b]
        kf_sb = kpool.tile([128, 2, 25, nb], FP, tag="kf")
        # all-25-tap diag for the last batch, built right after conv1
        diag_last = kpool.tile([128, 2, 25, 128], F16, tag="diag_last")

        def load_deferred_consts():
            # issued after conv2(0)'s matmuls so these don't compete with the
            # search/ws DMAs for HBM during the kernel head
            for cg in range(2):
                nc.gpsimd.dma_start(out=k_sbs[cg][:], in_=kin[:, cg])
            nc.gpsimd.dma_start(out=wk_sb[:], in_=wk_d[:])
            nc.gpsimd.dma_start(out=wf_sb[:], in_=wf_d[:])

        def conv1():
            for hg in range(2):
                ps = ps_c.tile([128, 5, 5, nb], FP, tag="psc")
                n_mm = 0
                for cg in range(2):
                    for dy in range(3):
                        for dx in range(3):
                            t = dy * 3 + dx
                            nc.tensor.matmul(
                                ps[:],
                                lhsT=wk_sb[:, hg * 18 + t * 2 + cg, :],
                                rhs=k_sbs[cg][:, dy:dy + 5, dx:dx + 5, :],
                                start=(n_mm == 0),
                                stop=(n_mm == 17),
                            )
                            n_mm += 1
                nc.scalar.activation(
                    out=kf_sb[:, hg, :, :],
                    in_=ps.rearrange("p a b c -> p (a b) c"),
                    func=RELU,
                    bias=bias_sb[:, 0 + hg:1 + hg],
                    scale=1.0,
                )

        def conv3_and_store(b, corr):
            out_sb = opool.tile([128, 2, 25, 25], F16, tag="osb")
            for og in range(2):
                for (r0, nr) in O_SPLITS:
                    ps = ps_o.tile([128, nr, 25], FP, tag="pso")
                    for hg in range(2):
                        nc.tensor.matmul(
                            ps[:],
                            lhsT=wf_sb[:, hg * 2 + og, :],
                            rhs=corr[:, hg, r0:r0 + nr, 0:25],
                            start=(hg == 0),
                            stop=(hg == 1),
                        )
                    nc.scalar.activation(
                        out=out_sb[:, og, r0:r0 + nr, :],
                        in_=ps[:],
                        func=RELU,
                        bias=bias_sb[:, 4 + og:5 + og],
                        scale=1.0,
                    )
                # the last two batches' og1 stores ride the scalar queue:
                # at the tail nothing else uses it, and splitting the two
                # final stores across queues shortens the drain
                dq = nc.scalar if (og == 1 and b >= nb - 2) else nc.sync
                dq.dma_start(
                    out=out_d[b, og * 128:(og + 1) * 128, :, :],
                    in_=out_sb[:, og, :, :],
                )

        def xcorr_pe(b, sf, diag, acc, last=False):
            # PE taps accumulate in PSUM, then the offloaded partials are
            # added with an identity matmul; relu epilogue merges to corr.
            # For the last batch (acc None) all 25 taps run on the PE.
            corr = cpool.tile([128, 2, 25, 32], F16, tag="corr")
            if last:
                taps, dix = PE_LAST, [ti * 5 + tj for (ti, tj) in PE_LAST]
            else:
                taps, dix = PE_TAPS, list(range(len(PE_TAPS)))
            for hg in range(2):
                for (r0, nr) in O_SPLITS:
                    ps = ps_x.tile([128, nr, 25], FP, tag="psx")
                    for i, (ti, tj) in enumerate(taps):
                        nc.tensor.matmul(
                            ps[:],
                            lhsT=diag[:, hg, dix[i], :],
                            rhs=sf[:, hg, ti + r0:ti + r0 + nr, tj:tj + 25],
                            start=(i == 0),
                            stop=False,
                        )
                    nc.tensor.matmul(
                        ps[:], lhsT=eye_sb[:],
                        rhs=acc[:, hg, r0:r0 + nr, 0:25],
                        start=False, stop=True,
                    )
                    nc.scalar.activation(
                        out=corr[:, hg, r0:r0 + nr, 0:25],
                        in_=ps[:],
                        func=RELU,
                        scale=1.0,
                    )
            return corr

        # --- per-batch main pipeline, software-pipelined with lag 2: the PE
        # xcorr + conv3 for batch b-2 are emitted after batch b's conv2 and
        # offloaded taps, so the PE never waits on the slower xcorr engines. ---
        state = {}
        sf_store = {}
        fp8_set = set(FP8_TAPS)
        for b in range(nb):
            if b + 1 < nb:
                load_search(b + 1)
            s_sb, s8_sb = s_tiles.pop(b)

            # conv2: search branch -> sf [h_part, hg, 29, 30] (col 29 unused
            # pad); 3 taps as fp8 DoubleRow (contract both cg in one matmul),
            # 6 taps fp16
            sf = fpool.tile([128, 2, 29, 32], F16, tag="sf")
            for hg in range(2):
                for (y0, ny) in C2_SPLITS:
                    ps = ps_c.tile([128, ny, 32], FP, tag="psc")
                    n_mm = 0
                    for cg in range(2):
                        for dy in range(3):
                            for dx in range(3):
                                if (dy, dx) in fp8_set:
                                    continue
                                t = dy * 3 + dx
                                n_mm += 1
                                nc.tensor.matmul(
                                    ps[:, :, 0:29],
                                    lhsT=ws_sb[:, hg * 18 + t * 2 + cg, :],
                                    rhs=s_sb[
                                        :, cg, dy + y0:dy + y0 + ny, dx:dx + 29
                                    ],
                                    start=(n_mm == 1),
                                    stop=False,
                                )
                    for i8, (dy, dx) in enumerate(FP8_TAPS):
                        nc.tensor.matmul(
                            ps[:],
                            lhsT=ws8_sb[:, hg * 3 + i8, :, :],
                            rhs=s8_sb[:, :, dy + y0:dy + y0 + ny, 0:32],
                            start=False,
                            stop=(i8 == len(FP8_TAPS) - 1),
                            perf_mode=DR,
                        )
                    nc.scalar.activation(
                        out=sf[:, hg, y0:y0 + ny, 0:29],
                        in_=ps[:, :, 0:29],
                        func=RELU,
                        bias=bias_sb[:, 2 + hg:3 + hg],
                        scale=1.0,
                    )
            if b == 0:
                load_deferred_consts()
                conv1()
                # the last batch's 25-tap diag only needs kf: build it now,
                # while the DVE is otherwise idle, so the tail never waits
                # on it
                for half in range(2):
                    nc.vector.tensor_mul(
                        diag_last[:, :, half * 13:half * 13 + 13 - half, :],
                        kf_sb[:, :, half * 13:half * 13 + 13 - half, nb - 1]
                        .unsqueeze(3).broadcast_to([128, 2, 13 - half, 128]),
                        eye_sb.unsqueeze(1).unsqueeze(1)
                        .broadcast_to([128, 2, 13 - half, 128]),
                    )

            sf_store[b] = sf

            def emit_offload(k):
                # diag + offloaded taps for batch k (reads sf(k))
                sfk = sf_store[k]
                diag = dpool.tile([128, 2, len(PE_TAPS), 128], F16, tag="diag")
                nr = PE_NRUN
                nc.vector.tensor_mul(
                    diag[:, :, 0:nr, :],
                    kf_sb[:, :, PE_T0:PE_T0 + nr, k]
                    .unsqueeze(3).broadcast_to([128, 2, nr, 128]),
                    eye_sb.unsqueeze(1).unsqueeze(1)
                    .broadcast_to([128, 2, nr, 128]),
                )
                acc = apool.tile([128, 2, 25, 32], F16, tag="acc")
                for i, (ti, tj) in enumerate(AV_TAPS):
                    t = ti * 5 + tj
                    dst = acc if i == 0 else tpool.tile(
                        [128, 2, 25, 32], F16, tag="tmp")
                    for hg in range(2):
                        nc.scalar.activation(
                            out=dst[:, hg, :, 0:25],
                            in_=sfk[:, hg, ti:ti + 25, tj:tj + 25],
                            func=COPY,
                            scale=kf_sb[:, hg, t, k:k + 1],
                        )
                    if i > 0:
                        nc.vector.tensor_add(
                            acc[:, :, :, 0:25], acc[:, :, :, 0:25],
                            dst[:, :, :, 0:25]
                        )
                for (ti, tj) in TS_TAPS:
                    t = ti * 5 + tj
                    tmp = tpool.tile([128, 2, 25, 32], F16, tag="tmp")
                    for hg in range(2):
                        nc.vector.tensor_scalar_mul(
                            tmp[:, hg, :, 0:25],
                            sfk[:, hg, ti:ti + 25, tj:tj + 25],
                            kf_sb[:, hg, t, k:k + 1],
                        )
                    nc.vector.tensor_add(
                        acc[:, :, :, 0:25], acc[:, :, :, 0:25], tmp[:, :, :, 0:25]
                    )
                state[k] = (sf_store.pop(k), diag, acc)

            # xcorr-PE + conv3 for batch b-2 go ahead of this batch's offload
            # mults in every engine queue, so the PE's dependencies (the relu
            # epilogues on the scalar engine) are never stuck behind them.
            # The offload must be emitted in the SAME iteration as its conv2:
            # deferring it an iteration puts the AV copies behind the next
            # batch's conv2 relus in the ACT queue, serializing the chain
            # against the next batch's PE work (measured +55us).
            if b >= 2:
                corr = xcorr_pe(b - 2, *state.pop(b - 2))
                conv3_and_store(b - 2, corr)
            if b < nb - 1:
                emit_offload(b)
            if b == nb - 1:
                # shorten the tail: batch nb-2 runs at lag 1 (its offload
                # finished during this batch's conv2) and ahead of this
                # batch's xcorr in every engine queue.
                corr = xcorr_pe(b - 1, *state.pop(b - 1))
                conv3_and_store(b - 1, corr)
                # last batch: 6 AV taps on the (tail-idle) scalar engine,
                # 19 PE taps from the prebuilt diag
                acc = apool.tile([128, 2, 25, 32], F16, tag="acc")
                for i, (ti, tj) in enumerate(AV_LAST):
                    t = ti * 5 + tj
                    dst = acc if i == 0 else tpool.tile(
                        [128, 2, 25, 32], F16, tag="tmp")
                    for hg in range(2):
                        nc.scalar.activation(
                            out=dst[:, hg, :, 0:25],
                            in_=sf[:, hg, ti:ti + 25, tj:tj + 25],
                            func=COPY,
                            scale=kf_sb[:, hg, t, b:b + 1],
                        )
                    if i > 0:
                        nc.vector.tensor_add(
                            acc[:, :, :, 0:25], acc[:, :, :, 0:25],
                            dst[:, :, :, 0:25]
                        )
                state[b] = (sf_store.pop(b), diag_last, acc)
        corr = xcorr_pe(nb - 1, *state.pop(nb - 1), last=True)
        conv3_and_store(nb - 1, corr)

    nc.compile()
    return nc


def _fold_bn(W, g, be, m, v):
    inv = (g.astype(np.float64) / np.sqrt(v.astype(np.float64) + EPS))
    Wp = (W.astype(np.float64) * inv[:, None, None, None]).astype(np.float32)
    bp = (be.astype(np.float64) - m.astype(np.float64) * inv).astype(np.float32)
    return Wp, bp


def _q8(x):
    return np.clip(np.asarray(x, np.float32), -240, 240).astype(
        ml_dtypes.float8_e4m3)


def _pack_weights(Wk, gk, bk, mk, vk, Ws, gs, bs, ms, vs, Wf, gf, bf, mf, vf):
    Wkp, bkp = _fold_bn(Wk, gk, bk, mk, vk)
    Wsp, bsp = _fold_bn(Ws, gs, bs, ms, vs)
    Wfp, bfp = _fold_bn(Wf, gf, bf, mf, vf)

    def pack33(Wp):  # [H, C, 3, 3] -> [k, (hg, t, cg), m]
        w = Wp.reshape(2, 128, 2, 128, 3, 3)  # hg, m, cg, k, dy, dx
        w = w.transpose(3, 0, 4, 5, 2, 1)  # k, hg, dy, dx, cg, m
        return np.ascontiguousarray(w.reshape(128, 36, 128))

    wk_h = pack33(Wkp).astype(np.float16)
    ws_f32 = pack33(Wsp)
    ws_h = ws_f32.astype(np.float16)
    # fp8 DoubleRow weights for FP8_TAPS: [k, hg*3+i, cg, m]
    ws8_h = np.zeros((128, 6, 2, 128), np.float32)
    for hg in range(2):
        for i, (dy, dx) in enumerate(FP8_TAPS):
            t = dy * 3 + dx
            for cg in range(2):
                ws8_h[:, hg * 3 + i, cg, :] = ws_f32[:, hg * 18 + t * 2 + cg, :]
    ws8_h = _q8(ws8_h)
    w = Wfp[:, :, 0, 0].reshape(2, 128, 2, 128)  # og, m, hg, k
    wf_h = np.ascontiguousarray(
        w.transpose(3, 2, 0, 1).reshape(128, 4, 128)).astype(np.float16)

    bias_h = np.zeros((128, 6), np.float32)
    bias_h[:, 0] = bkp[0:128]
    bias_h[:, 1] = bkp[128:256]
    bias_h[:, 2] = bsp[0:128]
    bias_h[:, 3] = bsp[128:256]
    bias_h[:, 4] = bfp[0:128]
    bias_h[:, 5] = bfp[128:256]
    eye_h = np.eye(128, dtype=np.float16)
    return wk_h, ws_h, ws8_h, wf_h, bias_h, eye_h


_NC_CACHE = {}


def _get_nc(nb):
    if nb not in _NC_CACHE:
        _NC_CACHE[nb] = _build_nc(nb)
    return _NC_CACHE[nb]


def run(inputs, trace=False):
    """Build in_maps, run on 8 cores, return (full_output, BassKernelResults)."""
    kernel = np.asarray(inputs["kernel"], np.float32)
    search = np.asarray(inputs["search"], np.float32)
    wk_h, ws_h, ws8_h, wf_h, bias_h, eye_h = _pack_weights(
        np.asarray(inputs["Wk"]), np.asarray(inputs["gk"]), np.asarray(inputs["bk"]),
        np.asarray(inputs["mk"]), np.asarray(inputs["vk"]),
        np.asarray(inputs["Ws"]), np.asarray(inputs["gs"]), np.asarray(inputs["bs"]),
        np.asarray(inputs["ms"]), np.asarray(inputs["vs"]),
        np.asarray(inputs["Wf"]), np.asarray(inputs["gf"]), np.asarray(inputs["bf"]),
        np.asarray(inputs["mf"]), np.asarray(inputs["vf"]),
    )
    nc = _get_nc(NB)
    # fp16/fp8 on host: identical to the on-device cast the kernel would do
    search_p = np.zeros((B, C, 31, 32), np.float16)
    search_p[:, :, :, :31] = search
    search8_p = np.zeros((B, C, 31, 32), ml_dtypes.float8_e4m3)
    search8_p[:, :, :, :31] = _q8(search)
    in_maps = []
    for i in range(N_CORES):
        kk = kernel[i * NB:(i + 1) * NB].reshape(NB, 2, 128, 7, 7)
        kin_h = np.ascontiguousarray(kk.transpose(2, 1, 3, 4, 0)).astype(np.float16)
        in_maps.append({
            "search": np.ascontiguousarray(search_p[i * NB:(i + 1) * NB]),
            "search8": np.ascontiguousarray(search8_p[i * NB:(i + 1) * NB]),
            "kin": kin_h,
            "wk": wk_h, "ws": ws_h, "ws8": ws8_h, "wf": wf_h,
            "bias": bias_h, "eye": eye_h,
        })
    res = run_bass_kernel_spmd(
        nc, in_maps, core_ids=list(range(N_CORES)), trace=trace
    )
    out = np.concatenate(
        [res.results[i]["out"].astype(np.float32) for i in range(N_CORES)], axis=0)
    return out, res


def kernel(**inputs):
    out, _ = run(inputs, trace=False)
    return out
